# revision 8
# baseline (speedup 1.0000x reference)
"""Trainium2 Bass kernel for supervised contrastive loss over N=8192 rows.

Strategy (8-core SPMD, rows sharded 1024/core):
  - Per column chunk t (128 cols): simT[c, r] = emb_cols_t @ emb_rows.T via PE
    (fp32r, full rate), exp(sim/T) on the scalar engine (bf16 out), diagonal
    zeroed by an off-diag mask multiply, then S_T[class, row] += onehotT @ exp
    on PE (bf16).  Classes partition the columns, so total_sum = sum_c S_T and
    positive_sum = sum_c S_T * onehotR.  Tiny ones-matmul + log tail produces
    per-row masked loss; host sums partials and divides by the valid count.
  - The diagonal's chunk position is made core-invariant by rotating each
    core's column-side data (embeddings and one-hots) by its row offset.
"""

import os
import numpy as np
import ml_dtypes

import concourse.tile as tile
from concourse import bacc, mybir
from concourse.bass_utils import run_bass_kernel_spmd

N, D, C = 8192, 128, 100
NCORES = 8
R = N // NCORES  # rows per core
NT = N // 128  # column chunks of 128
TEMP = 0.07
F32 = mybir.dt.float32
F32R = mybir.dt.float32r
BF16 = mybir.dt.bfloat16

_PROGRAM_CACHE = {}


def _build_program(mm1_dt):
    nc = bacc.Bacc("TRN2", target_bir_lowering=False, debug=False, num_devices=NCORES)

    embT_cols = nc.dram_tensor("embT_cols", [D, N], F32, kind="ExternalInput")
    embT_rows = nc.dram_tensor("embT_rows", [D, R], F32, kind="ExternalInput")
    ohc = nc.dram_tensor("ohc", [N, C], BF16, kind="ExternalInput")
    ohrT = nc.dram_tensor("ohrT", [C, R], F32, kind="ExternalInput")
    negval = nc.dram_tensor("negval", [1, R], F32, kind="ExternalInput")
    offdiag = nc.dram_tensor("offdiag", [128, 128], BF16, kind="ExternalInput")
    out = nc.dram_tensor("out", [1, R], F32, kind="ExternalOutput")

    with tile.TileContext(nc) as tc:
        with (
            tc.tile_pool(name="consts", bufs=1) as consts,
            tc.tile_pool(name="spool", bufs=1, space="PSUM") as spool,
            tc.tile_pool(name="simpool", bufs=2, space="PSUM") as simpool,
            tc.tile_pool(name="exppool", bufs=3) as exppool,
            tc.tile_pool(name="fpool", bufs=1, space="PSUM") as fpool,
            tc.tile_pool(name="fsb", bufs=1) as fsb,
        ):
            # Resident inputs. Column-side embeddings split into 8 tiles so the
            # first matmuls don't wait for the whole 4MB load.
            cols_sb = []
            for j in range(8):
                tcol = consts.tile([D, 1024], mm1_dt, tag=f"col{j}")
                nc.sync.dma_start(
                    tcol[:], embT_cols[:, j * 1024 : (j + 1) * 1024].bitcast(mm1_dt)
                )
                cols_sb.append(tcol)
            rows_sb = consts.tile([D, R], mm1_dt, tag="rows")
            nc.sync.dma_start(rows_sb[:], embT_rows[:, :].bitcast(mm1_dt))

            ohc_sb = consts.tile([128, NT, C], BF16, tag="ohc")
            ohc_re = ohc[:, :].rearrange("(t p) c -> p t c", p=128)
            for j in range(8):
                sl = slice(j * 8, (j + 1) * 8)
                nc.sync.dma_start(ohc_sb[:, sl, :], ohc_re[:, sl, :])

            offd_sb = consts.tile([128, 128], BF16, tag="offd")
            nc.sync.dma_start(offd_sb[:], offdiag[:, :])
            ohrT_sb = consts.tile([C, R], F32, tag="ohrT")
            nc.sync.dma_start(ohrT_sb[:], ohrT[:, :])
            nv_sb = consts.tile([1, R], F32, tag="nv")
            nc.sync.dma_start(nv_sb[:], negval[:, :])
            ones_sb = consts.tile([C, 1], F32, tag="ones")
            nc.vector.memset(ones_sb[:], 1.0)

            # S_T[class, row] accumulator over all column chunks. Split into
            # two 512-row tiles: a matmul output must stay within one PSUM bank.
            S_T = [
                spool.tile([C, 512], F32, tag=f"S{q}", name=f"S_T{q}")
                for q in range(2)
            ]

            # Software-pipelined main loop: sim+exp for chunk t+1 are issued
            # before the accumulation matmul of chunk t, so the PE keeps
            # working while the scalar engine computes exp.
            sim_tiles = [None] * NT
            exp_tiles = [None] * NT

            def emit_sim_exp(t):
                sim_ps = simpool.tile([128, R], F32)
                lhsT = cols_sb[t // 8][:, (t % 8) * 128 : (t % 8 + 1) * 128]
                for h in range(2):
                    sl = slice(h * 512, (h + 1) * 512)
                    nc.tensor.matmul(
                        sim_ps[:, sl],
                        lhsT,
                        rows_sb[:, sl],
                        start=True,
                        stop=True,
                    )
                exp_sb = exppool.tile([128, R], BF16)
                nc.scalar.activation(
                    exp_sb[:], sim_ps[:], mybir.ActivationFunctionType.Exp,
                    scale=float(1.0 / TEMP),
                )
                if t < 8:
                    # Chunk t's columns are rows t*128..t*128+127 of this core:
                    # the diagonal is the main diagonal of this 128x128 block.
                    blk = slice(t * 128, (t + 1) * 128)
                    nc.vector.tensor_mul(exp_sb[:, blk], exp_sb[:, blk], offd_sb[:])
                sim_tiles[t] = sim_ps
                exp_tiles[t] = exp_sb

            emit_sim_exp(0)
            for t in range(NT):
                if t + 1 < NT:
                    emit_sim_exp(t + 1)
                for q in range(2):
                    nc.tensor.matmul(
                        S_T[q][:],
                        ohc_sb[:, t, :],
                        exp_tiles[t][:, q * 512 : (q + 1) * 512],
                        start=(t == 0),
                        stop=(t == NT - 1),
                    )

            # Final: per-row total / positive sums then masked -log.
            S_sb = fsb.tile([C, R], F32, tag="S_sb")
            for q in range(2):
                nc.vector.tensor_copy(S_sb[:, q * 512 : (q + 1) * 512], S_T[q][:])
            posS = fsb.tile([C, R], F32, tag="posS")
            nc.vector.tensor_mul(posS[:], S_sb[:], ohrT_sb[:])
            out_sb = fsb.tile([1, R], F32, tag="out_sb")
            for q in range(2):
                sl = slice(q * 512, (q + 1) * 512)
                tot_ps = fpool.tile([1, 512], F32)
                nc.tensor.matmul(tot_ps[:], ones_sb[:], S_sb[:, sl], start=True, stop=True)
                pos_ps = fpool.tile([1, 512], F32)
                nc.tensor.matmul(pos_ps[:], ones_sb[:], posS[:, sl], start=True, stop=True)
                t1 = fsb.tile([1, 512], F32, tag="t1")
                nc.vector.tensor_scalar_add(t1[:], tot_ps[:], 1e-8)
                rec = fsb.tile([1, 512], F32, tag="rec")
                nc.vector.reciprocal(rec[:], t1[:])
                ratio = fsb.tile([1, 512], F32, tag="ratio")
                nc.vector.tensor_mul(ratio[:], pos_ps[:], rec[:])
                nc.vector.tensor_scalar_add(ratio[:], ratio[:], 1e-8)
                lg = fsb.tile([1, 512], F32, tag="lg")
                nc.scalar.activation(lg[:], ratio[:], mybir.ActivationFunctionType.Ln)
                nc.vector.tensor_mul(out_sb[:, sl], lg[:], nv_sb[:, sl])
            nc.sync.dma_start(out[:, :], out_sb[:])

    nc.compile()
    return nc


def _get_program():
    mm1 = os.environ.get("CONTRASTIVE_MM1_DT", "f32r")
    mm1_dt = F32R if mm1 == "f32r" else F32
    key = mm1
    if key not in _PROGRAM_CACHE:
        _PROGRAM_CACHE[key] = _build_program(mm1_dt)
    return _PROGRAM_CACHE[key]


def _prepare_in_maps(embeddings, labels):
    emb = np.asarray(embeddings, dtype=np.float32)
    lab = np.asarray(labels).astype(np.int64)
    embT = np.ascontiguousarray(emb.T)  # [D, N]
    classes = np.arange(C, dtype=np.int64)
    onehot = lab[:, None] == classes[None, :]  # [N, C] bool
    oh_bf16 = onehot.astype(ml_dtypes.bfloat16)
    oh_f32 = onehot.astype(np.float32)
    counts = np.bincount(lab, minlength=C)
    valid = (counts[lab] - 1) > 0  # [N] bool
    negval = np.where(valid, -1.0, 0.0).astype(np.float32)
    offd = (1.0 - np.eye(128, dtype=np.float32)).astype(ml_dtypes.bfloat16)

    in_maps = []
    for i in range(NCORES):
        r0 = i * R
        in_maps.append(
            {
                "embT_cols": np.ascontiguousarray(np.roll(embT, -r0, axis=1)),
                "embT_rows": np.ascontiguousarray(embT[:, r0 : r0 + R]),
                "ohc": np.ascontiguousarray(np.roll(oh_bf16, -r0, axis=0)),
                "ohrT": np.ascontiguousarray(oh_f32[r0 : r0 + R].T),
                "negval": np.ascontiguousarray(negval[r0 : r0 + R][None, :]),
                "offdiag": offd,
            }
        )
    return in_maps, valid


def run(embeddings, labels, trace=False, trace_cores=None):
    """Returns (mean_loss, BassKernelResults)."""
    in_maps, valid = _prepare_in_maps(embeddings, labels)
    nc = _get_program()
    kwargs = {}
    if trace:
        kwargs["trace"] = True
        if trace_cores is not None:
            kwargs["trace_cores"] = trace_cores
    res = run_bass_kernel_spmd(nc, in_maps, core_ids=list(range(NCORES)), **kwargs)
    loss_sum = 0.0
    for i in range(NCORES):
        loss_sum += float(res.results[i]["out"].astype(np.float64).sum())
    cnt = int(valid.sum())
    mean = loss_sum / cnt if cnt > 0 else 0.0
    return np.asarray(mean, dtype=np.float32), res


def kernel(embeddings, labels):
    return run(embeddings, labels)[0]


# revision 9
# speedup vs baseline: 1.2339x; 1.2339x over previous
"""Trainium2 Bass kernel for supervised contrastive loss over N=8192 rows.

Strategy (8-core SPMD, rows sharded 1024/core):
  - Per column chunk t (128 cols): simT[c, r] = emb_cols_t @ emb_rows.T via PE,
    exp(sim/T) on the scalar engine (bf16 out), diagonal zeroed by an off-diag
    mask multiply, then S_T[class, row] += onehot_colsT @ exp on PE (bf16).
    Classes partition the columns, so total_sum = sum_c S_T and positive_sum =
    sum_c S_T * onehotR.  A small per-row-chunk matmul tail produces per-row
    masked loss; the host sums partials and divides by the valid count.
  - The diagonal's chunk position is made core-invariant by rotating each
    core's column-side data (embeddings and one-hots) by its row offset.
"""

import os
import numpy as np
import ml_dtypes

import concourse.tile as tile
from concourse import bacc, mybir
from concourse.bass_utils import run_bass_kernel_spmd

N, D, C = 8192, 128, 100
NCORES = 8
R = N // NCORES  # rows per core
NT = N // 128  # column chunks of 128
RC = R // 128  # row chunks per core (8)
TEMP = 0.07
F32 = mybir.dt.float32
F32R = mybir.dt.float32r
F16 = mybir.dt.float16
BF16 = mybir.dt.bfloat16

_PROGRAM_CACHE = {}


def _build_program(mm1_mode):
    mm1_dt = {"f16": F16, "f32r": F32R, "f32": F32}[mm1_mode]
    emb_np_dt = np.float16 if mm1_mode == "f16" else np.float32

    nc = bacc.Bacc("TRN2", target_bir_lowering=False, debug=False, num_devices=NCORES)

    emb_dram_dt = F16 if mm1_mode == "f16" else F32
    embT_cols = nc.dram_tensor("embT_cols", [D, N], emb_dram_dt, kind="ExternalInput")
    embT_rows = nc.dram_tensor("embT_rows", [D, R], emb_dram_dt, kind="ExternalInput")
    ohc = nc.dram_tensor("ohc", [N, C], BF16, kind="ExternalInput")
    ohrT = nc.dram_tensor("ohrT", [C, R], F32, kind="ExternalInput")
    negval = nc.dram_tensor("negval", [128, RC], F32, kind="ExternalInput")
    offdiag = nc.dram_tensor("offdiag", [128, 128], BF16, kind="ExternalInput")
    out = nc.dram_tensor("out", [128, RC], F32, kind="ExternalOutput")

    def bc(ap):
        # view a DRAM fp32 AP as fp32r when needed so the verifier sees
        # fp32r-typed producers for fp32r matmuls
        return ap.bitcast(F32R) if mm1_mode == "f32r" else ap

    with tile.TileContext(nc) as tc:
        with (
            tc.tile_pool(name="consts", bufs=1) as consts,
            tc.tile_pool(name="spool", bufs=1, space="PSUM") as spool,
            tc.tile_pool(name="simpool", bufs=2, space="PSUM") as simpool,
            tc.tile_pool(name="exppool", bufs=3) as exppool,
            tc.tile_pool(name="fpool", bufs=1, space="PSUM") as fpool,
            tc.tile_pool(name="fsb", bufs=1) as fsb,
        ):
            # Resident inputs, ordered so the first chunk's dependencies land
            # first: rows, cols[0], ohc[0], then the rest streams behind
            # compute.
            rows_sb = consts.tile([D, R], mm1_dt, tag="rows")
            nc.sync.dma_start(rows_sb[:], bc(embT_rows[:, :]))

            cols_sb = []
            for j in range(8):
                tcol = consts.tile([D, 1024], mm1_dt, tag=f"col{j}", name=f"cols_sb{j}")
                cols_sb.append(tcol)
            ohc_sb = consts.tile([128, NT, C], BF16, tag="ohc")
            ohc_re = ohc[:, :].rearrange("(t p) c -> p t c", p=128)

            def load_chunk(j):
                nc.sync.dma_start(
                    cols_sb[j][:], bc(embT_cols[:, j * 1024 : (j + 1) * 1024])
                )
                sl = slice(j * 8, (j + 1) * 8)
                nc.sync.dma_start(ohc_sb[:, sl, :], ohc_re[:, sl, :])

            load_chunk(0)
            offd_sb = consts.tile([128, 128], BF16, tag="offd")
            nc.sync.dma_start(offd_sb[:], offdiag[:, :])
            for j in range(1, 8):
                load_chunk(j)
            ohrT_sb = consts.tile([C, R], F32, tag="ohrT")
            nc.sync.dma_start(ohrT_sb[:], ohrT[:, :])
            nv_sb = consts.tile([128, RC], F32, tag="nv")
            nc.sync.dma_start(nv_sb[:], negval[:, :])
            ones_sb = consts.tile([C, 1], F32, tag="ones")
            nc.vector.memset(ones_sb[:], 1.0)

            # S_T[class, row] accumulator over all column chunks. Split into
            # two 512-row tiles: a matmul output must stay within one PSUM bank.
            S_T = [
                spool.tile([C, 512], F32, tag=f"S{q}", name=f"S_T{q}")
                for q in range(2)
            ]

            # Software-pipelined main loop: sim+exp for chunk t+1 are issued
            # before the accumulation matmuls of chunk t, so the PE keeps
            # working while the scalar engine computes exp.
            exp_tiles = [None] * NT

            def emit_sim_exp(t):
                sim_ps = simpool.tile([128, R], F32, name=f"sim{t}", tag="sim")
                lhsT = cols_sb[t // 8][:, (t % 8) * 128 : (t % 8 + 1) * 128]
                for h in range(2):
                    sl = slice(h * 512, (h + 1) * 512)
                    nc.tensor.matmul(
                        sim_ps[:, sl], lhsT, rows_sb[:, sl], start=True, stop=True
                    )
                exp_sb = exppool.tile([128, R], BF16, name=f"exp{t}", tag="exp")
                nc.scalar.activation(
                    exp_sb[:], sim_ps[:], mybir.ActivationFunctionType.Exp,
                    scale=float(1.0 / TEMP),
                )
                if t < RC:
                    # Chunk t's columns are rows t*128..t*128+127 of this core:
                    # the diagonal is the main diagonal of this 128x128 block.
                    blk = slice(t * 128, (t + 1) * 128)
                    nc.vector.tensor_mul(exp_sb[:, blk], exp_sb[:, blk], offd_sb[:])
                exp_tiles[t] = exp_sb

            emit_sim_exp(0)
            for t in range(NT):
                if t + 1 < NT:
                    emit_sim_exp(t + 1)
                for q in range(2):
                    nc.tensor.matmul(
                        S_T[q][:],
                        ohc_sb[:, t, :],
                        exp_tiles[t][:, q * 512 : (q + 1) * 512],
                        start=(t == 0),
                        stop=(t == NT - 1),
                    )

            # Tail: per-row totals via small matmuls so everything stays in a
            # [128, RC] layout (row = chunk*128 + partition).
            S_sb = fsb.tile([C, R], F32, tag="S_sb")
            for q in range(2):
                nc.vector.tensor_copy(S_sb[:, q * 512 : (q + 1) * 512], S_T[q][:])
            posS = fsb.tile([C, R], F32, tag="posS")
            nc.vector.tensor_mul(posS[:], S_sb[:], ohrT_sb[:])

            tot_ps = fpool.tile([128, RC], F32, tag="tot")
            pos_ps = fpool.tile([128, RC], F32, tag="pos")
            for j in range(RC):
                sl = slice(j * 128, (j + 1) * 128)
                nc.tensor.matmul(
                    tot_ps[:, j : j + 1], S_sb[:, sl], ones_sb[:], start=True, stop=True
                )
                nc.tensor.matmul(
                    pos_ps[:, j : j + 1], posS[:, sl], ones_sb[:], start=True, stop=True
                )
            t1 = fsb.tile([128, RC], F32, tag="t1")
            nc.vector.tensor_scalar_add(t1[:], tot_ps[:], 1e-8)
            rec = fsb.tile([128, RC], F32, tag="rec")
            nc.vector.reciprocal(rec[:], t1[:])
            ratio = fsb.tile([128, RC], F32, tag="ratio")
            nc.vector.tensor_mul(ratio[:], pos_ps[:], rec[:])
            nc.vector.tensor_scalar_add(ratio[:], ratio[:], 1e-8)
            lg = fsb.tile([128, RC], F32, tag="lg")
            nc.scalar.activation(lg[:], ratio[:], mybir.ActivationFunctionType.Ln)
            out_sb = fsb.tile([128, RC], F32, tag="out_sb")
            nc.vector.tensor_mul(out_sb[:], lg[:], nv_sb[:])
            nc.sync.dma_start(out[:, :], out_sb[:])

    nc.compile()
    return nc, emb_np_dt


def _get_program():
    mm1_mode = os.environ.get("CONTRASTIVE_MM1_DT", "f16")
    if mm1_mode not in _PROGRAM_CACHE:
        _PROGRAM_CACHE[mm1_mode] = _build_program(mm1_mode)
    return _PROGRAM_CACHE[mm1_mode]


def _prepare_in_maps(embeddings, labels, emb_np_dt):
    emb = np.asarray(embeddings, dtype=np.float32)
    lab = np.asarray(labels).astype(np.int64)
    embT = np.ascontiguousarray(emb.T).astype(emb_np_dt)  # [D, N]
    classes = np.arange(C, dtype=np.int64)
    onehot = lab[:, None] == classes[None, :]  # [N, C] bool
    oh_bf16 = onehot.astype(ml_dtypes.bfloat16)
    oh_f32 = onehot.astype(np.float32)
    counts = np.bincount(lab, minlength=C)
    valid = (counts[lab] - 1) > 0  # [N] bool
    negval = np.where(valid, -1.0, 0.0).astype(np.float32)
    offd = (1.0 - np.eye(128, dtype=np.float32)).astype(ml_dtypes.bfloat16)

    in_maps = []
    for i in range(NCORES):
        r0 = i * R
        in_maps.append(
            {
                "embT_cols": np.ascontiguousarray(np.roll(embT, -r0, axis=1)),
                "embT_rows": np.ascontiguousarray(embT[:, r0 : r0 + R]),
                "ohc": np.ascontiguousarray(np.roll(oh_bf16, -r0, axis=0)),
                "ohrT": np.ascontiguousarray(oh_f32[r0 : r0 + R].T),
                # [128, RC] with row = chunk*128 + partition
                "negval": np.ascontiguousarray(negval[r0 : r0 + R].reshape(RC, 128).T),
                "offdiag": offd,
            }
        )
    return in_maps, valid


def run(embeddings, labels, trace=False, trace_cores=None):
    """Returns (mean_loss, BassKernelResults)."""
    nc, emb_np_dt = _get_program()
    in_maps, valid = _prepare_in_maps(embeddings, labels, emb_np_dt)
    kwargs = {}
    if trace:
        kwargs["trace"] = True
        if trace_cores is not None:
            kwargs["trace_cores"] = trace_cores
    res = run_bass_kernel_spmd(nc, in_maps, core_ids=list(range(NCORES)), **kwargs)
    loss_sum = 0.0
    for i in range(NCORES):
        loss_sum += float(res.results[i]["out"].astype(np.float64).sum())
    cnt = int(valid.sum())
    mean = loss_sum / cnt if cnt > 0 else 0.0
    return np.asarray(mean, dtype=np.float32), res


def kernel(embeddings, labels):
    return run(embeddings, labels)[0]


# revision 11
# speedup vs baseline: 1.2511x; 1.0139x over previous
"""Trainium2 Bass kernel for supervised contrastive loss over N=8192 rows.

Strategy (8-core SPMD, rows sharded 1024/core):
  - Per column chunk t (128 cols): simT[c, r] = emb_cols_t @ emb_rows.T via PE,
    exp(sim/T) on the scalar engine (bf16 out), diagonal zeroed by an off-diag
    mask multiply, then S_T[class, row] += onehot_colsT @ exp on PE (bf16).
    Classes partition the columns, so total_sum = sum_c S_T and positive_sum =
    sum_c S_T * onehotR.  A small per-row-chunk matmul tail produces per-row
    masked loss; the host sums partials and divides by the valid count.
  - The diagonal's chunk position is made core-invariant by rotating each
    core's column-side data (embeddings and one-hots) by its row offset.
"""

import os
import numpy as np
import ml_dtypes

import concourse.tile as tile
from concourse import bacc, mybir
from concourse.bass_utils import run_bass_kernel_spmd

N, D, C = 8192, 128, 100
NCORES = 8
R = N // NCORES  # rows per core
NT = N // 128  # column chunks of 128
RC = R // 128  # row chunks per core (8)
TEMP = 0.07
F32 = mybir.dt.float32
F32R = mybir.dt.float32r
F16 = mybir.dt.float16
BF16 = mybir.dt.bfloat16

_PROGRAM_CACHE = {}


def _build_program(mm1_mode):
    mm1_dt = {"f16": F16, "bf16": BF16, "f32r": F32R, "f32": F32}[mm1_mode]
    emb_np_dt = {
        "f16": np.float16,
        "bf16": ml_dtypes.bfloat16,
        "f32r": np.float32,
        "f32": np.float32,
    }[mm1_mode]

    nc = bacc.Bacc("TRN2", target_bir_lowering=False, debug=False, num_devices=NCORES)

    emb_dram_dt = {"f16": F16, "bf16": BF16, "f32r": F32, "f32": F32}[mm1_mode]
    embT_cols = nc.dram_tensor("embT_cols", [D, N], emb_dram_dt, kind="ExternalInput")
    embT_rows = nc.dram_tensor("embT_rows", [D, R], emb_dram_dt, kind="ExternalInput")
    ohc = nc.dram_tensor("ohc", [N, C], BF16, kind="ExternalInput")
    ohrT = nc.dram_tensor("ohrT", [C, R], F32, kind="ExternalInput")
    negval = nc.dram_tensor("negval", [128, RC], F32, kind="ExternalInput")
    offdiag = nc.dram_tensor("offdiag", [128, 128], BF16, kind="ExternalInput")
    out = nc.dram_tensor("out", [128, RC], F32, kind="ExternalOutput")

    def bc(ap):
        # view a DRAM fp32 AP as fp32r when needed so the verifier sees
        # fp32r-typed producers for fp32r matmuls
        return ap.bitcast(F32R) if mm1_mode == "f32r" else ap

    with tile.TileContext(nc) as tc:
        with (
            tc.tile_pool(name="consts", bufs=1) as consts,
            tc.tile_pool(name="spool", bufs=1, space="PSUM") as spool,
            tc.tile_pool(name="simpool", bufs=2, space="PSUM") as simpool,
            tc.tile_pool(name="exppool", bufs=3) as exppool,
            tc.tile_pool(name="fpool", bufs=1, space="PSUM") as fpool,
            tc.tile_pool(name="fsb", bufs=1) as fsb,
        ):
            # Resident inputs, ordered so the first chunk's dependencies land
            # first: rows, cols[0], ohc[0], then the rest streams behind
            # compute.
            rows_sb = consts.tile([D, R], mm1_dt, tag="rows")
            nc.sync.dma_start(rows_sb[:], bc(embT_rows[:, :]))

            cols_sb = []
            for j in range(8):
                tcol = consts.tile([D, 1024], mm1_dt, tag=f"col{j}", name=f"cols_sb{j}")
                cols_sb.append(tcol)
            ohc_sb = consts.tile([128, NT, C], BF16, tag="ohc")
            ohc_re = ohc[:, :].rearrange("(t p) c -> p t c", p=128)

            def load_chunk(j):
                nc.sync.dma_start(
                    cols_sb[j][:], bc(embT_cols[:, j * 1024 : (j + 1) * 1024])
                )
                sl = slice(j * 8, (j + 1) * 8)
                nc.sync.dma_start(ohc_sb[:, sl, :], ohc_re[:, sl, :])

            load_chunk(0)
            offd_sb = consts.tile([128, 128], BF16, tag="offd")
            nc.sync.dma_start(offd_sb[:], offdiag[:, :])
            for j in range(1, 8):
                load_chunk(j)
            ohrT_sb = consts.tile([C, R], F32, tag="ohrT")
            nc.sync.dma_start(ohrT_sb[:], ohrT[:, :])
            nv_sb = consts.tile([128, RC], F32, tag="nv")
            nc.sync.dma_start(nv_sb[:], negval[:, :])
            ones_sb = consts.tile([C, 1], F32, tag="ones")
            nc.vector.memset(ones_sb[:], 1.0)

            # S_T[class, row] accumulator over all column chunks. Split into
            # two 512-row tiles: a matmul output must stay within one PSUM bank.
            S_T = [
                spool.tile([C, 512], F32, tag=f"S{q}", name=f"S_T{q}")
                for q in range(2)
            ]

            # Software-pipelined main loop: sim+exp for chunk t+1 are issued
            # before the accumulation matmuls of chunk t, so the PE keeps
            # working while the scalar engine computes exp.
            exp_tiles = [None] * NT

            def emit_sim_exp(t):
                sim_ps = simpool.tile([128, R], F32, name=f"sim{t}", tag="sim")
                lhsT = cols_sb[t // 8][:, (t % 8) * 128 : (t % 8 + 1) * 128]
                for h in range(2):
                    sl = slice(h * 512, (h + 1) * 512)
                    nc.tensor.matmul(
                        sim_ps[:, sl], lhsT, rows_sb[:, sl], start=True, stop=True
                    )
                exp_sb = exppool.tile([128, R], BF16, name=f"exp{t}", tag="exp")
                nc.scalar.activation(
                    exp_sb[:], sim_ps[:], mybir.ActivationFunctionType.Exp,
                    scale=float(1.0 / TEMP),
                )
                if t < RC:
                    # Chunk t's columns are rows t*128..t*128+127 of this core:
                    # the diagonal is the main diagonal of this 128x128 block.
                    blk = slice(t * 128, (t + 1) * 128)
                    nc.vector.tensor_mul(exp_sb[:, blk], exp_sb[:, blk], offd_sb[:])
                exp_tiles[t] = exp_sb

            emit_sim_exp(0)
            for t in range(NT):
                if t + 1 < NT:
                    emit_sim_exp(t + 1)
                for q in range(2):
                    nc.tensor.matmul(
                        S_T[q][:],
                        ohc_sb[:, t, :],
                        exp_tiles[t][:, q * 512 : (q + 1) * 512],
                        start=(t == 0),
                        stop=(t == NT - 1),
                    )

            # Tail: per-row totals via small matmuls so everything stays in a
            # [128, RC] layout (row = chunk*128 + partition).
            S_sb = fsb.tile([C, R], F32, tag="S_sb")
            for q in range(2):
                nc.vector.tensor_copy(S_sb[:, q * 512 : (q + 1) * 512], S_T[q][:])
            posS = fsb.tile([C, R], F32, tag="posS")
            nc.vector.tensor_mul(posS[:], S_sb[:], ohrT_sb[:])

            tot_ps = fpool.tile([128, RC], F32, tag="tot")
            pos_ps = fpool.tile([128, RC], F32, tag="pos")
            for j in range(RC):
                sl = slice(j * 128, (j + 1) * 128)
                nc.tensor.matmul(
                    tot_ps[:, j : j + 1], S_sb[:, sl], ones_sb[:], start=True, stop=True
                )
                nc.tensor.matmul(
                    pos_ps[:, j : j + 1], posS[:, sl], ones_sb[:], start=True, stop=True
                )
            t1 = fsb.tile([128, RC], F32, tag="t1")
            nc.vector.tensor_scalar_add(t1[:], tot_ps[:], 1e-8)
            rec = fsb.tile([128, RC], F32, tag="rec")
            nc.vector.reciprocal(rec[:], t1[:])
            ratio = fsb.tile([128, RC], F32, tag="ratio")
            nc.vector.tensor_mul(ratio[:], pos_ps[:], rec[:])
            nc.vector.tensor_scalar_add(ratio[:], ratio[:], 1e-8)
            lg = fsb.tile([128, RC], F32, tag="lg")
            nc.scalar.activation(lg[:], ratio[:], mybir.ActivationFunctionType.Ln)
            out_sb = fsb.tile([128, RC], F32, tag="out_sb")
            nc.vector.tensor_mul(out_sb[:], lg[:], nv_sb[:])
            nc.sync.dma_start(out[:, :], out_sb[:])

    nc.compile()
    return nc, emb_np_dt


def _get_program():
    mm1_mode = os.environ.get("CONTRASTIVE_MM1_DT", "f16")
    if mm1_mode not in _PROGRAM_CACHE:
        _PROGRAM_CACHE[mm1_mode] = _build_program(mm1_mode)
    return _PROGRAM_CACHE[mm1_mode]


def _prepare_in_maps(embeddings, labels, emb_np_dt):
    emb = np.asarray(embeddings, dtype=np.float32)
    lab = np.asarray(labels).astype(np.int64)
    embT = np.ascontiguousarray(emb.T).astype(emb_np_dt)  # [D, N]
    classes = np.arange(C, dtype=np.int64)
    onehot = lab[:, None] == classes[None, :]  # [N, C] bool
    oh_bf16 = onehot.astype(ml_dtypes.bfloat16)
    oh_f32 = onehot.astype(np.float32)
    counts = np.bincount(lab, minlength=C)
    valid = (counts[lab] - 1) > 0  # [N] bool
    negval = np.where(valid, -1.0, 0.0).astype(np.float32)
    offd = (1.0 - np.eye(128, dtype=np.float32)).astype(ml_dtypes.bfloat16)

    in_maps = []
    for i in range(NCORES):
        r0 = i * R
        in_maps.append(
            {
                "embT_cols": np.ascontiguousarray(np.roll(embT, -r0, axis=1)),
                "embT_rows": np.ascontiguousarray(embT[:, r0 : r0 + R]),
                "ohc": np.ascontiguousarray(np.roll(oh_bf16, -r0, axis=0)),
                "ohrT": np.ascontiguousarray(oh_f32[r0 : r0 + R].T),
                # [128, RC] with row = chunk*128 + partition
                "negval": np.ascontiguousarray(negval[r0 : r0 + R].reshape(RC, 128).T),
                "offdiag": offd,
            }
        )
    return in_maps, valid


def run(embeddings, labels, trace=False, trace_cores=None):
    """Returns (mean_loss, BassKernelResults)."""
    nc, emb_np_dt = _get_program()
    in_maps, valid = _prepare_in_maps(embeddings, labels, emb_np_dt)
    kwargs = {}
    if trace:
        kwargs["trace"] = True
        if trace_cores is not None:
            kwargs["trace_cores"] = trace_cores
    res = run_bass_kernel_spmd(nc, in_maps, core_ids=list(range(NCORES)), **kwargs)
    loss_sum = 0.0
    for i in range(NCORES):
        loss_sum += float(res.results[i]["out"].astype(np.float64).sum())
    cnt = int(valid.sum())
    mean = loss_sum / cnt if cnt > 0 else 0.0
    return np.asarray(mean, dtype=np.float32), res


def kernel(embeddings, labels):
    return run(embeddings, labels)[0]


# revision 18
# speedup vs baseline: 1.2848x; 1.0270x over previous
"""Trainium2 Bass kernel for supervised contrastive loss over N=8192 rows.

Strategy (8-core SPMD, rows sharded 1024/core):
  - Per column chunk t (128 cols): simT[c, r] = emb_cols_t @ emb_rows.T via PE,
    exp(sim/T) on the scalar engine (bf16 out), diagonal zeroed by an off-diag
    mask multiply, then S_T[class, row] += onehot_colsT @ exp on PE (bf16).
    Classes partition the columns, so total_sum = sum_c S_T and positive_sum =
    sum_c S_T * onehotR.  A small per-row-chunk matmul tail produces per-row
    masked loss; the host sums partials and divides by the valid count.
  - The diagonal's chunk position is made core-invariant by rotating each
    core's column-side data (embeddings and one-hots) by its row offset.
"""

import os
import numpy as np
import ml_dtypes

import concourse.tile as tile
from concourse import bacc, mybir
from concourse.bass_utils import run_bass_kernel_spmd

N, D, C = 8192, 128, 100
NCORES = 8
R = N // NCORES  # rows per core
NT = N // 128  # column chunks of 128
RC = R // 128  # row chunks per core (8)
TEMP = 0.07
F32 = mybir.dt.float32
F32R = mybir.dt.float32r
F16 = mybir.dt.float16
BF16 = mybir.dt.bfloat16

_PROGRAM_CACHE = {}


def _build_program(mm1_mode):
    mm1_dt = {"f16": F16, "bf16": BF16, "f32r": F32R, "f32": F32}[mm1_mode]
    emb_np_dt = {
        "f16": np.float16,
        "bf16": ml_dtypes.bfloat16,
        "f32r": np.float32,
        "f32": np.float32,
    }[mm1_mode]

    nc = bacc.Bacc("TRN2", target_bir_lowering=False, debug=False, num_devices=NCORES)

    emb_dram_dt = {"f16": F16, "bf16": BF16, "f32r": F32, "f32": F32}[mm1_mode]
    embT_cols = nc.dram_tensor("embT_cols", [D, N], emb_dram_dt, kind="ExternalInput")
    embT_rows = nc.dram_tensor("embT_rows", [D, R], emb_dram_dt, kind="ExternalInput")
    ohc = nc.dram_tensor("ohc", [N, C], BF16, kind="ExternalInput")
    ohrT = nc.dram_tensor("ohrT", [C, R], BF16, kind="ExternalInput")
    negval = nc.dram_tensor("negval", [128, RC], F32, kind="ExternalInput")
    offdiag = nc.dram_tensor("offdiag", [128, 128], BF16, kind="ExternalInput")
    out = nc.dram_tensor("out", [128, RC], F32, kind="ExternalOutput")

    def bc(ap):
        # view a DRAM fp32 AP as fp32r when needed so the verifier sees
        # fp32r-typed producers for fp32r matmuls
        return ap.bitcast(F32R) if mm1_mode == "f32r" else ap

    with tile.TileContext(nc) as tc:
        with (
            tc.tile_pool(name="consts", bufs=1) as consts,
            tc.tile_pool(name="spool", bufs=1, space="PSUM") as spool,
            tc.tile_pool(name="simpool", bufs=2, space="PSUM") as simpool,
            tc.tile_pool(name="exppool", bufs=3) as exppool,
            tc.tile_pool(name="fpool", bufs=1, space="PSUM") as fpool,
            tc.tile_pool(name="fsb", bufs=1) as fsb,
        ):
            # Resident inputs, ordered so the first chunk's dependencies land
            # first: rows, cols[0], ohc[0], then the rest streams behind
            # compute.
            # Critical-path loads first, in small pieces, so chunk 0's matmul
            # and accumulation unblock within ~1us of kernel start.
            rows_sb = consts.tile([D, R], mm1_dt, tag="rows")
            cols_sb = []
            for j in range(8):
                tcol = consts.tile([D, 1024], mm1_dt, tag=f"col{j}", name=f"cols_sb{j}")
                cols_sb.append(tcol)
            ohc_sb = consts.tile([128, NT, C], BF16, tag="ohc")
            ohc_re = ohc[:, :].rearrange("(t p) c -> p t c", p=128)

            nc.sync.dma_start(cols_sb[0][:, 0:128], bc(embT_cols[:, 0:128]))
            nc.sync.dma_start(rows_sb[:, 0:512], bc(embT_rows[:, 0:512]))
            nc.sync.dma_start(rows_sb[:, 512:R], bc(embT_rows[:, 512:R]))
            nc.sync.dma_start(ohc_sb[:, 0:1, :], ohc_re[:, 0:1, :])
            offd_sb = consts.tile([128, 128], BF16, tag="offd")
            nc.sync.dma_start(offd_sb[:], offdiag[:, :])
            nc.sync.dma_start(cols_sb[0][:, 128:1024], bc(embT_cols[:, 128:1024]))
            nc.sync.dma_start(ohc_sb[:, 1:8, :], ohc_re[:, 1:8, :])
            for j in range(1, 8):
                nc.sync.dma_start(
                    cols_sb[j][:], bc(embT_cols[:, j * 1024 : (j + 1) * 1024])
                )
                sl = slice(j * 8, (j + 1) * 8)
                nc.sync.dma_start(ohc_sb[:, sl, :], ohc_re[:, sl, :])
            ohrT_sb = consts.tile([C, R], BF16, tag="ohrT")
            nc.sync.dma_start(ohrT_sb[:], ohrT[:, :])
            nv_sb = consts.tile([128, RC], F32, tag="nv")
            nc.sync.dma_start(nv_sb[:], negval[:, :])

            # S_T[class, row] accumulator over all column chunks. Split into
            # two 512-row tiles: a matmul output must stay within one PSUM bank.
            S_T = [
                spool.tile([C, 512], F32, tag=f"S{q}", name=f"S_T{q}")
                for q in range(2)
            ]

            # Software-pipelined main loop: sim+exp for chunk t+1 are issued
            # before the accumulation matmuls of chunk t, so the PE keeps
            # working while the scalar engine computes exp.
            exp_tiles = [None] * NT

            def emit_sim_exp(t):
                sim_ps = simpool.tile([128, R], F32, name=f"sim{t}", tag="sim")
                lhsT = cols_sb[t // 8][:, (t % 8) * 128 : (t % 8 + 1) * 128]
                for h in range(2):
                    sl = slice(h * 512, (h + 1) * 512)
                    nc.tensor.matmul(
                        sim_ps[:, sl], lhsT, rows_sb[:, sl], start=True, stop=True
                    )
                exp_sb = exppool.tile([128, R], BF16, name=f"exp{t}", tag="exp")
                nc.scalar.activation(
                    exp_sb[:], sim_ps[:], mybir.ActivationFunctionType.Exp,
                    scale=float(1.0 / TEMP),
                )
                if t < RC:
                    # Chunk t's columns are rows t*128..t*128+127 of this core:
                    # the diagonal is the main diagonal of this 128x128 block.
                    blk = slice(t * 128, (t + 1) * 128)
                    nc.vector.tensor_mul(exp_sb[:, blk], exp_sb[:, blk], offd_sb[:])
                exp_tiles[t] = exp_sb

            emit_sim_exp(0)
            for t in range(NT):
                if t + 1 < NT:
                    emit_sim_exp(t + 1)
                for q in range(2):
                    nc.tensor.matmul(
                        S_T[q][:],
                        ohc_sb[:, t, :],
                        exp_tiles[t][:, q * 512 : (q + 1) * 512],
                        start=(t == 0),
                        stop=(t == NT - 1),
                    )

            # Tail: per-row totals via small matmuls so everything stays in a
            # [128, RC] layout (row = chunk*128 + partition).
            S_sb = fsb.tile([C, R], BF16, tag="S_sb")
            for q in range(2):
                nc.vector.tensor_copy(S_sb[:, q * 512 : (q + 1) * 512], S_T[q][:])
            posS = fsb.tile([C, R], BF16, tag="posS")
            nc.vector.tensor_mul(posS[:], S_sb[:], ohrT_sb[:])

            tot_ps = fpool.tile([128, RC], F32, tag="tot")
            pos_ps = fpool.tile([128, RC], F32, tag="pos")
            ones_bf = consts.tile([C, 1], BF16, tag="ones_bf")
            nc.vector.memset(ones_bf[:], 1.0)
            for j in range(RC):
                sl = slice(j * 128, (j + 1) * 128)
                nc.tensor.matmul(
                    tot_ps[:, j : j + 1], S_sb[:, sl], ones_bf[:], start=True, stop=True
                )
                nc.tensor.matmul(
                    pos_ps[:, j : j + 1], posS[:, sl], ones_bf[:], start=True, stop=True
                )
            t1 = fsb.tile([128, RC], F32, tag="t1")
            nc.vector.tensor_scalar_add(t1[:], tot_ps[:], 1e-8)
            rec = fsb.tile([128, RC], F32, tag="rec")
            nc.vector.reciprocal(rec[:], t1[:])
            ratio = fsb.tile([128, RC], F32, tag="ratio")
            nc.vector.tensor_mul(ratio[:], pos_ps[:], rec[:])
            nc.vector.tensor_scalar_add(ratio[:], ratio[:], 1e-8)
            lg = fsb.tile([128, RC], F32, tag="lg")
            nc.scalar.activation(lg[:], ratio[:], mybir.ActivationFunctionType.Ln)
            out_sb = fsb.tile([128, RC], F32, tag="out_sb")
            nc.vector.tensor_mul(out_sb[:], lg[:], nv_sb[:])
            nc.sync.dma_start(out[:, :], out_sb[:])

    nc.compile()
    return nc, emb_np_dt


def _get_program():
    mm1_mode = os.environ.get("CONTRASTIVE_MM1_DT", "f16")
    if mm1_mode not in _PROGRAM_CACHE:
        _PROGRAM_CACHE[mm1_mode] = _build_program(mm1_mode)
    return _PROGRAM_CACHE[mm1_mode]


def _prepare_in_maps(embeddings, labels, emb_np_dt):
    emb = np.asarray(embeddings, dtype=np.float32)
    lab = np.asarray(labels).astype(np.int64)
    embT = np.ascontiguousarray(emb.T).astype(emb_np_dt)  # [D, N]
    classes = np.arange(C, dtype=np.int64)
    onehot = lab[:, None] == classes[None, :]  # [N, C] bool
    oh_bf16 = onehot.astype(ml_dtypes.bfloat16)
    counts = np.bincount(lab, minlength=C)
    valid = (counts[lab] - 1) > 0  # [N] bool
    negval = np.where(valid, -1.0, 0.0).astype(np.float32)
    offd = (1.0 - np.eye(128, dtype=np.float32)).astype(ml_dtypes.bfloat16)

    in_maps = []
    for i in range(NCORES):
        r0 = i * R
        in_maps.append(
            {
                "embT_cols": np.ascontiguousarray(np.roll(embT, -r0, axis=1)),
                "embT_rows": np.ascontiguousarray(embT[:, r0 : r0 + R]),
                "ohc": np.ascontiguousarray(np.roll(oh_bf16, -r0, axis=0)),
                "ohrT": np.ascontiguousarray(oh_bf16[r0 : r0 + R].T),
                # [128, RC] with row = chunk*128 + partition
                "negval": np.ascontiguousarray(negval[r0 : r0 + R].reshape(RC, 128).T),
                "offdiag": offd,
            }
        )
    return in_maps, valid


def run(embeddings, labels, trace=False, trace_cores=None):
    """Returns (mean_loss, BassKernelResults)."""
    nc, emb_np_dt = _get_program()
    in_maps, valid = _prepare_in_maps(embeddings, labels, emb_np_dt)
    kwargs = {}
    if trace:
        kwargs["trace"] = True
        if trace_cores is not None:
            kwargs["trace_cores"] = trace_cores
    res = run_bass_kernel_spmd(nc, in_maps, core_ids=list(range(NCORES)), **kwargs)
    loss_sum = 0.0
    for i in range(NCORES):
        loss_sum += float(res.results[i]["out"].astype(np.float64).sum())
    cnt = int(valid.sum())
    mean = loss_sum / cnt if cnt > 0 else 0.0
    return np.asarray(mean, dtype=np.float32), res


def kernel(embeddings, labels):
    return run(embeddings, labels)[0]
